# revision 1
# baseline (speedup 1.0000x reference)
"""Trainium2 Bass kernel for per-view cross-attention.

Reference computation (per view v of 1024, S=64 samples, D=256):
  qp = q @ Wq.T + pe ; kp = k @ Wk.T + pe ; vp = v @ Wv.T + pe
  attn = softmax(qp @ kp.T / sqrt(D))
  x = gelu(attn @ vp @ Wo.T + bo) + q
Sharding: data-parallel over the 1024 views across 8 cores (128 views each).

On-chip layout strategy: everything is kept in "transposed" space [D, rows]
(rows = view*64+s) so that the contraction dim D lands on SBUF partitions
without any on-chip input transposes. The host pre-transposes q/k/v shards to
[D, rows] (free: numpy) and post-transposes the [D, rows] output back.
v additionally needs its projected form in natural [row, D] layout for the
attn@v matmul; that drops out naturally by using vT as the matmul stationary.
"""

import sys
import os

for p in ("/opt/trn_rl_repo",):
    if p not in sys.path and os.path.isdir(p):
        sys.path.insert(0, p)

import numpy as np

V, S, D = 1024, 64, 256
N_CORES = 8
VC = V // N_CORES          # views per core
ROWS = VC * S              # 8192 rows per core
R = 512                    # rows per supertile (8 views)
NST = ROWS // R            # supertiles per core
NV = R // S                # views per supertile
GELU_GROUP = 4             # supertiles per gelu flush (ACT table amortization)
PROJ_BUFS = 3
SM_BUFS = 3
PS_S_BUFS = 1
PS_T_BUFS = 1
PS_A_BUFS = 3
PS_B_BUFS = 3
LD_BUFS = 3
SCALE = 1.0 / np.sqrt(np.float32(D)).astype(np.float32)

_CACHE = {}


def _make_posenc(d_hid, n_samples):
    pos = np.arange(n_samples, dtype=np.float64)[:, None]
    j = np.arange(d_hid)[None, :]
    angle = pos / np.power(10000.0, 2.0 * (j // 2) / d_hid)
    table = np.where(j % 2 == 0, np.sin(angle), np.cos(angle))
    return table.astype(np.float32)  # [S, D]


def _build(rows=ROWS, stage=99):
    import concourse.bass as bass
    import concourse.mybir as mybir
    import concourse.tile as tile
    from concourse.tile import add_dep_helper
    from concourse import bacc
    from contextlib import ExitStack

    fp32 = mybir.dt.float32
    f32r = mybir.dt.float32r
    bf16 = mybir.dt.bfloat16
    AF = mybir.ActivationFunctionType
    ALU = mybir.AluOpType
    n_st = rows // R

    nc = bacc.Bacc(None, target_bir_lowering=False)

    qT_d = nc.dram_tensor("qT", [D, rows], f32r, kind="ExternalInput")
    kT_d = nc.dram_tensor("kT", [D, rows], f32r, kind="ExternalInput")
    vT_d = nc.dram_tensor("vT", [D, rows], f32r, kind="ExternalInput")
    wq_d = nc.dram_tensor("WqT", [D, D], f32r, kind="ExternalInput")
    wk_d = nc.dram_tensor("WkT", [D, D], f32r, kind="ExternalInput")
    wv_d = nc.dram_tensor("WvT", [D, D], f32r, kind="ExternalInput")
    wo_d = nc.dram_tensor("WoT", [D, D], f32r, kind="ExternalInput")
    bo_d = nc.dram_tensor("bo", [D], fp32, kind="ExternalInput")
    pet_d = nc.dram_tensor("peT_rep", [D, R], fp32, kind="ExternalInput")
    pe_d = nc.dram_tensor("pe_nat", [S, D], f32r, kind="ExternalInput")
    e2_d = nc.dram_tensor("E2", [S, 128], f32r, kind="ExternalInput")
    id_d = nc.dram_tensor("I128", [128, 128], fp32, kind="ExternalInput")
    out_d = nc.dram_tensor("outT", [D, rows], fp32, kind="ExternalOutput")

    def r3(ap):  # [D, X] dram -> [128, 2, X] partition view
        return ap.rearrange("(kc p) r -> p kc r", p=128)

    with tile.TileContext(nc) as tc, ExitStack() as ctx:
        const = ctx.enter_context(tc.tile_pool(name="const", bufs=1))
        ld = ctx.enter_context(tc.tile_pool(name="ld", bufs=LD_BUFS))
        proj = ctx.enter_context(tc.tile_pool(name="proj", bufs=PROJ_BUFS))
        sm = ctx.enter_context(tc.tile_pool(name="sm", bufs=SM_BUFS))
        psA = ctx.enter_context(tc.tile_pool(name="psA", bufs=PS_A_BUFS, space="PSUM"))
        psB = ctx.enter_context(tc.tile_pool(name="psB", bufs=PS_B_BUFS, space="PSUM"))
        psS = ctx.enter_context(tc.tile_pool(name="psS", bufs=PS_S_BUFS, space="PSUM"))
        psT = ctx.enter_context(tc.tile_pool(name="psT", bufs=PS_T_BUFS, space="PSUM"))
        stg = ctx.enter_context(tc.tile_pool(name="stg", bufs=GELU_GROUP + 1))

        wq = const.tile([128, 2, D], f32r)
        wk = const.tile([128, 2, D], f32r)
        wv = const.tile([128, 2, D], f32r)
        wo = const.tile([128, 2, D], f32r)
        nc.sync.dma_start(wq, r3(wq_d[:]))
        nc.sync.dma_start(wk, r3(wk_d[:]))
        nc.sync.dma_start(wv, r3(wv_d[:]))
        nc.sync.dma_start(wo, r3(wo_d[:]))
        pet = const.tile([128, 2, R], fp32)
        nc.sync.dma_start(pet, r3(pet_d[:]))
        pe_sb = const.tile([S, D], f32r)
        nc.sync.dma_start(pe_sb, pe_d[:])
        e2 = const.tile([S, 128], f32r)
        nc.sync.dma_start(e2, e2_d[:])
        i128 = const.tile([128, 128], fp32)
        nc.sync.dma_start(i128, id_d[:])
        bo_sb = const.tile([128, 2], fp32)
        nc.sync.dma_start(bo_sb, bo_d.rearrange("(kc p) -> p kc", p=128))

        pending = []
        last_gelu = None
        last_exp = None
        for st in range(n_st):
            rs = slice(st * R, (st + 1) * R)
            qt = ld.tile([128, 2, R], f32r, tag="qt", bufs=GELU_GROUP + 2)
            kt = ld.tile([128, 2, R], f32r, tag="kt")
            vt = ld.tile([128, 2, R], f32r, tag="vt")
            nc.sync.dma_start(qt, r3(qT_d[:])[:, :, rs])
            nc.sync.dma_start(kt, r3(kT_d[:])[:, :, rs])
            nc.sync.dma_start(vt, r3(vT_d[:])[:, :, rs])

            # ---- projections into transposed space: xpT[dout, row] ----
            qpT = proj.tile([128, 2, R], fp32, tag="qpT")
            kpT = proj.tile([128, 2, R], fp32, tag="kpT")
            for w_sb, x_sb, o_sb in ((wq, qt, qpT), (wk, kt, kpT)):
                for mc in range(2):
                    ps = psA.tile([128, R], fp32, tag="psA", name="ps_proj")
                    for kc in range(2):
                        nc.tensor.matmul(
                            ps,
                            w_sb[:, kc, mc * 128:(mc + 1) * 128],
                            x_sb[:, kc, :],
                            start=(kc == 0),
                            stop=(kc == 1),
                        )
                    # evacuate PSUM fused with positional-encoding add
                    nc.vector.tensor_add(
                        out=o_sb[:, mc, :], in0=ps, in1=pet[:, mc, :]
                    )

            if stage <= 1:
                nc.sync.dma_start(r3(out_d[:])[:, :, rs], qpT)
                continue
            # ---- vp in natural [row, dout] layout (vT as stationary) ----
            vp = proj.tile([128, 4, D], fp32, tag="vp")
            for g in range(4):
                psv = psB.tile([128, D], fp32, tag="psB", name="ps_vp")
                for kc in range(2):
                    nc.tensor.matmul(
                        psv,
                        vt[:, kc, g * 128:(g + 1) * 128],
                        wv[:, kc, :],
                        start=(kc == 0),
                        stop=False,
                    )
                # pe add folded in as a matmul: E2.T @ pe = pe tiled over rows
                nc.tensor.matmul(psv, e2, pe_sb, start=False, stop=True)
                nc.scalar.copy(out=vp[:, g, :], in_=psv)

            if stage <= 2:
                nc.sync.dma_start(r3(out_d[:])[:, :, rs], vp.rearrange("p a b -> p (a b)")[:, None, :].rearrange("p o (a b) -> p (o a) b", a=2))
                continue
            # ---- scores: per view [64,64], packed [128(2 views), 4, 64] ----
            scps = psS.tile([128, 4, S], fp32, tag="scores")
            for v in range(NV):
                g, h = v // 2, v % 2
                for dc in range(2):
                    nc.tensor.matmul(
                        scps[h * 64:(h + 1) * 64, g, :],
                        qpT[:, dc, v * S:(v + 1) * S],
                        kpT[:, dc, v * S:(v + 1) * S],
                        start=(dc == 0),
                        stop=(dc == 1),
                        tile_position=(0, h * 64),
                    )

            # ---- softmax along free axis (no max-subtraction: |scores/16|<~10) ----
            attn = sm.tile([128, 4, S], fp32, tag="attn")
            _e = nc.scalar.activation(attn, scps, AF.Exp, scale=float(SCALE))
            # keep Exp-set ops contiguous on ACT: exp of a new gelu-group must
            # come after the previous group's last gelu
            if last_gelu is not None:
                add_dep_helper(_e.ins, last_gelu, sync=False,
                               reason="act-table grouping: exp after prior gelus")
            last_exp = _e.ins
            sums = sm.tile([128, 4], fp32, tag="sums")
            nc.vector.tensor_reduce(out=sums, in_=attn, axis=mybir.AxisListType.X, op=ALU.add)
            rec = sm.tile([128, 4], fp32, tag="rec")
            nc.vector.reciprocal(rec, sums)
            nc.vector.tensor_tensor(
                attn, attn, rec[:, :, None].to_broadcast((128, 4, S)), ALU.mult
            )

            if stage <= 3:
                nc.sync.dma_start(r3(out_d[:])[:, 0, st * R: st * R + 256], attn.rearrange("p a b -> p (a b)"))
                continue
            # ---- transpose attn packs; duplicate into both partition halves ----
            atps = psT.tile([128, 4, 128], fp32, tag="attnT")
            for g in range(4):
                for h in range(2):
                    nc.tensor.matmul(
                        atps[h * 64:(h + 1) * 64, g, :],
                        attn[:, g, :],
                        i128,
                        start=True,
                        stop=True,
                        tile_position=(0, h * 64),
                    )
            attnT = sm.tile([128, 4, 128], fp32, tag="attnT_sb")
            nc.scalar.copy(out=attnT, in_=atps)

            if stage <= 4:
                nc.sync.dma_start(r3(out_d[:])[:, 0, st * R: st * R + 512], attnT.rearrange("p a b -> p (a b)"))
                continue
            # ---- attn @ vp, directly in transposed space outT[d, row] ----
            # Concurrent row-group matmuls must not drain into the same
            # (partition, bank) pair: one PSUM tile per row-half h.
            outT = proj.tile([128, 2, R], f32r, tag="outT")
            for c in range(2):
                for h in range(2):
                    pso = psB.tile([128, 4, S], fp32, tag="psB", name="ps_av")
                    for g in range(4):
                        nc.tensor.matmul(
                            pso[:, g, :],
                            vp[h * 64:(h + 1) * 64, g, c * 128:(c + 1) * 128],
                            attnT[h * 64:(h + 1) * 64, g, h * 64:(h + 1) * 64],
                            start=True,
                            stop=True,
                            tile_position=(h * 64, 0),
                        )
                    # view v=2g+h lives at free offset v*64 of outT chunk c
                    o_ap = outT[:, c, :].rearrange(
                        "p (g two s) -> p g two s", two=2, s=S
                    )[:, :, h, :]
                    if c == 0:
                        nc.vector.tensor_copy(o_ap, pso)
                    else:
                        nc.scalar.copy(out=o_ap, in_=pso)

            if stage <= 5:
                nc.sync.dma_start(r3(out_d[:])[:, :, rs], outT)
                continue
            # ---- final projection, staged pre-gelu (Exp and Gelu live in
            # different ACT table sets; group gelus to amortize ~2.7us
            # table switches) ----
            pre = stg.tile([128, 2, R], fp32, tag="pre")
            for mc in range(2):
                psf = psA.tile([128, R], fp32, tag="psA", name="ps_fin")
                for kc in range(2):
                    nc.tensor.matmul(
                        psf,
                        wo[:, kc, mc * 128:(mc + 1) * 128],
                        outT[:, kc, :],
                        start=(kc == 0),
                        stop=(kc == 1),
                    )
                if mc == 0:
                    nc.vector.tensor_copy(pre[:, mc, :], psf)
                else:
                    nc.scalar.copy(out=pre[:, mc, :], in_=psf)
            pending.append((st, pre, qt))

            if len(pending) == GELU_GROUP or st == n_st - 1:
                for pst, ppre, pqt in pending:
                    outsb = proj.tile([128, 2, R], fp32, tag="outsb")
                    for mc in range(2):
                        _g = nc.scalar.activation(
                            out=outsb[:, mc, :], in_=ppre[:, mc, :],
                            func=AF.Gelu, bias=bo_sb[:, mc:mc + 1], scale=1.0,
                        )
                        if last_exp is not None:
                            add_dep_helper(_g.ins, last_exp, sync=False,
                                           reason="act-table grouping: gelu after group exps")
                        last_gelu = _g.ins
                        nc.vector.tensor_add(
                            out=outsb[:, mc, :], in0=outsb[:, mc, :],
                            in1=pqt[:, mc, :],
                        )
                    nc.sync.dma_start(
                        r3(out_d[:])[:, :, pst * R:(pst + 1) * R], outsb
                    )
                pending = []

    nc.finalize()
    return nc


def _get_nc():
    if "nc" not in _CACHE:
        _CACHE["nc"] = _build()
    return _CACHE["nc"]


def _host_inputs(q, k, v, Wq, Wk, Wv, Wo, bo):
    pe = _make_posenc(D, S)                      # [S, D]
    peT_rep = np.ascontiguousarray(np.tile(pe.T, (1, NV)))   # [D, R]
    e2 = np.ascontiguousarray(np.tile(np.eye(S, dtype=np.float32), (1, 2)))
    i128 = np.eye(128, dtype=np.float32)
    consts = {
        "WqT": np.ascontiguousarray(np.asarray(Wq, np.float32).T),
        "WkT": np.ascontiguousarray(np.asarray(Wk, np.float32).T),
        "WvT": np.ascontiguousarray(np.asarray(Wv, np.float32).T),
        "WoT": np.ascontiguousarray(np.asarray(Wo, np.float32).T),
        "bo": np.ascontiguousarray(np.asarray(bo, np.float32)),
        "peT_rep": peT_rep,
        "pe_nat": pe,
        "E2": e2,
        "I128": i128,
    }
    in_maps = []
    for c in range(N_CORES):
        sl = slice(c * VC, (c + 1) * VC)
        m = dict(consts)
        m["qT"] = np.ascontiguousarray(
            np.asarray(q, np.float32)[sl].reshape(ROWS, D).T)
        m["kT"] = np.ascontiguousarray(
            np.asarray(k, np.float32)[sl].reshape(ROWS, D).T)
        m["vT"] = np.ascontiguousarray(
            np.asarray(v, np.float32)[sl].reshape(ROWS, D).T)
        in_maps.append(m)
    return in_maps


def kernel(q, k, v, Wq, Wk, Wv, Wo, bo, _trace=False):
    from concourse.bass_utils import run_bass_kernel_spmd

    nc = _get_nc()
    in_maps = _host_inputs(q, k, v, Wq, Wk, Wv, Wo, bo)
    res = run_bass_kernel_spmd(nc, in_maps, list(range(N_CORES)), trace=_trace)
    outs = [
        res.results[c]["outT"].reshape(D, VC, S).transpose(1, 2, 0)
        for c in range(N_CORES)
    ]
    full = np.concatenate(outs, axis=0)
    if _trace:
        _CACHE["last_results"] = res
    return full



# revision 21
# speedup vs baseline: 2.0471x; 2.0471x over previous
"""Trainium2 Bass kernel for per-view cross-attention.

Reference computation (per view v of 1024, S=64 samples, D=256):
  qp = q @ Wq.T + pe ; kp = k @ Wk.T + pe ; vp = v @ Wv.T + pe
  attn = softmax(qp @ kp.T / sqrt(D))
  x = gelu(attn @ vp @ Wo.T + bo) + q
Sharding: data-parallel over the 1024 views across 8 cores (128 views each).

Layout: projections live in "transposed" space [D, rows] (rows = view*64+s)
so the contraction dim D lands on SBUF partitions with no on-chip
transposes; the host pre-transposes (free) and casts to bf16 (halves DMA;
bf16 moving operands run the small attention matmuls at 1 cyc/row vs 4).

Softmax runs in transposed space: scoresT[k, q] is computed directly by
swapping stationary/moving (kpT stationary, qpT moving), which removes the
attn-transpose matmuls entirely. The k (partition) sum uses a [128,2] ones
matmul into PSUM, reciprocal on DVE, and a [2,128] ones matmul to broadcast
1/sum back across partitions.

Engines execute their queues in order, so the loop is software-pipelined
with a two-stage skew: iteration t issues projections for supertile t, the
attention chain for u=t-1, and the outT evacuation + final projection for
w=t-2 (whose attn@v PSUM results waited overnight, so the copies run with
no latency on the critical path). Per-engine issue orders are arranged so
every engine has fill work while cross-engine dependencies resolve. Gelu
flushes are grouped (2 ACT table loads per group) and scheduled right after
the group's last pre-activation staging copy.
"""

import sys
import os

for p in ("/opt/trn_rl_repo",):
    if p not in sys.path and os.path.isdir(p):
        sys.path.insert(0, p)

import numpy as np

V, S, D = 1024, 64, 256
N_CORES = 8
VC = V // N_CORES          # views per core
ROWS = VC * S              # 8192 rows per core
R = 512                    # rows per supertile (8 views)
NST = ROWS // R            # supertiles per core
NV = R // S                # views per supertile
GELU_GROUP = 4             # supertiles per gelu flush (ACT table amortization)
SCALE = 1.0 / np.sqrt(np.float32(D)).astype(np.float32)

_CACHE = {}


def _make_posenc(d_hid, n_samples):
    pos = np.arange(n_samples, dtype=np.float64)[:, None]
    j = np.arange(d_hid)[None, :]
    angle = pos / np.power(10000.0, 2.0 * (j // 2) / d_hid)
    table = np.where(j % 2 == 0, np.sin(angle), np.cos(angle))
    return table.astype(np.float32)  # [S, D]


def _build(rows=ROWS):
    import concourse.bass as bass
    import concourse.mybir as mybir
    import concourse.tile as tile
    from concourse.tile import add_dep_helper
    from concourse import bacc
    from contextlib import ExitStack

    fp32 = mybir.dt.float32
    f32r = mybir.dt.float32r
    bf16 = mybir.dt.bfloat16
    AF = mybir.ActivationFunctionType
    ALU = mybir.AluOpType
    n_st = rows // R

    nc = bacc.Bacc(None, target_bir_lowering=False)

    qT_d = nc.dram_tensor("qT", [D, rows], bf16, kind="ExternalInput")
    kT_d = nc.dram_tensor("kT", [D, rows], bf16, kind="ExternalInput")
    vT_d = nc.dram_tensor("vT", [D, rows], bf16, kind="ExternalInput")
    wq_d = nc.dram_tensor("WqT", [D, D], bf16, kind="ExternalInput")
    wk_d = nc.dram_tensor("WkT", [D, D], bf16, kind="ExternalInput")
    wv_d = nc.dram_tensor("WvT", [D, D], bf16, kind="ExternalInput")
    wo_d = nc.dram_tensor("WoT", [D, D], bf16, kind="ExternalInput")
    bo_d = nc.dram_tensor("bo", [D], fp32, kind="ExternalInput")
    pet_d = nc.dram_tensor("peT_rep", [D, R], fp32, kind="ExternalInput")
    pe_d = nc.dram_tensor("pe_nat", [S, D], bf16, kind="ExternalInput")
    e2_d = nc.dram_tensor("E2", [S, 128], bf16, kind="ExternalInput")
    ob_d = nc.dram_tensor("ones_blk", [128, 2], bf16, kind="ExternalInput")
    o2_d = nc.dram_tensor("onesM", [2, 2, 128], f32r, kind="ExternalInput")
    out_d = nc.dram_tensor("outT", [D, rows], bf16, kind="ExternalOutput")

    def r3(ap):  # [D, X] dram -> [128, 2, X] partition view
        return ap.rearrange("(kc p) r -> p kc r", p=128)

    with tile.TileContext(nc) as tc, ExitStack() as ctx:
        const = ctx.enter_context(tc.tile_pool(name="const", bufs=1))
        ld = ctx.enter_context(tc.tile_pool(name="ld", bufs=3))
        proj = ctx.enter_context(tc.tile_pool(name="proj", bufs=3))
        sm = ctx.enter_context(tc.tile_pool(name="sm", bufs=3))
        psA = ctx.enter_context(tc.tile_pool(name="psA", bufs=3, space="PSUM"))
        psB = ctx.enter_context(tc.tile_pool(name="psB", bufs=3, space="PSUM"))
        psS = ctx.enter_context(tc.tile_pool(name="psS", bufs=1, space="PSUM"))
        psR = ctx.enter_context(tc.tile_pool(name="psR", bufs=1, space="PSUM"))
        stg = ctx.enter_context(tc.tile_pool(name="stg", bufs=GELU_GROUP + 3))

        # const loads, ordered so the first iteration's dependencies come in
        # first: wq/wk -> (qt/kt issued by caller right after) -> the rest.
        wq = const.tile([128, 2, D], bf16)
        wk = const.tile([128, 2, D], bf16)
        nc.sync.dma_start(wq, r3(wq_d[:]))

        st_state = {}       # per-supertile live tiles
        act_state = {"last_gelu": None, "last_exp": None}

        def unit_load(t):
            if not (0 <= t < n_st):
                return
            rs = slice(t * R, (t + 1) * R)
            s = st_state.setdefault(t, {})
            s["qt"] = ld.tile([128, 2, R], bf16, tag="qt", bufs=9, name="qt")
            s["kt"] = ld.tile([128, 2, R], bf16, tag="kt", name="kt")
            s["vt"] = ld.tile([128, 2, R], bf16, tag="vt", name="vt")
            nc.sync.dma_start(s["qt"], r3(qT_d[:])[:, :, rs])
            nc.sync.dma_start(s["kt"], r3(kT_d[:])[:, :, rs])
            nc.sync.dma_start(s["vt"], r3(vT_d[:])[:, :, rs])

        def unit_scores(u):
            # transposed scores scoresT[k, q] + exp -> attn (unnormalized)
            if not (0 <= u < n_st):
                return
            s = st_state[u]
            scps = psS.tile([128, 8, S], fp32, tag="scoresT", name="scps")
            for v in range(NV):
                g, h = v // 2, v % 2
                for dc in range(2):
                    nc.tensor.matmul(
                        scps[h * 64:(h + 1) * 64, g, :],
                        s["kpT"][:, dc, v * S:(v + 1) * S],
                        s["qpT"][:, dc, v * S:(v + 1) * S],
                        start=(dc == 0),
                        stop=(dc == 1),
                        tile_position=(0, h * 64),
                    )
            # softmax exp (no max-subtraction: |scores/16| < ~10)
            attn = sm.tile([128, 4, S], bf16, tag="attn", name="attn")
            _e = nc.scalar.activation(attn, scps[:, 0:4, :], AF.Exp, scale=float(SCALE))
            s["scps"] = scps
            if act_state["last_gelu"] is not None:
                add_dep_helper(_e.ins, act_state["last_gelu"], sync=False,
                               reason="act-table grouping: exp after prior gelus")
            act_state["last_exp"] = _e.ins
            s["attn"] = attn

        def unit_fc(w):
            # outT evacuation for stage w (pso PSUM filled last iteration):
            # chunk 0 on DVE, chunk 1 on ACT
            if not (0 <= w < n_st):
                return
            s = st_state[w]
            s["outT"] = proj.tile([128, 2, R], bf16, tag="outT", name="outT")
            nc.vector.tensor_copy(
                s["outT"][:, 0, :], s["pso0"].rearrange("p a b -> p (a b)")
            )
            nc.scalar.copy(
                out=s["outT"][:, 1, :], in_=s["pso1"].rearrange("p a b -> p (a b)")
            )

        def unit_vp1copy(tp):
            # ACT-side vp evacuation for the second vp chunk of stage t-1
            if not (0 <= tp < n_st):
                return
            s = st_state[tp]
            nc.scalar.copy(out=s["vp"][:, 2:4, :], in_=s["psv1"])

        def unit_proj_mm(t, which):
            # which: 0..3 -> (q,mc0),(q,mc1),(k,mc0),(k,mc1)
            if not (0 <= t < n_st):
                return
            s = st_state[t]
            isq, mc = which < 2, which % 2
            w_sb = wq if isq else wk
            x_sb = s["qt"] if isq else s["kt"]
            key = "qpT" if isq else "kpT"
            if mc == 0:
                s[key] = proj.tile([128, 2, R], bf16, tag=key, name=key)
            ps = psA.tile([128, R], fp32, tag="psA", name="ps_proj")
            for kc in range(2):
                nc.tensor.matmul(
                    ps,
                    w_sb[:, kc, mc * 128:(mc + 1) * 128],
                    x_sb[:, kc, :],
                    start=(kc == 0),
                    stop=(kc == 1),
                )
            s[f"pp{which}"] = ps

        def unit_proj_ev(t, which):
            # evacuate PSUM fused with positional-encoding add (DVE)
            if not (0 <= t < n_st):
                return
            s = st_state[t]
            isq, mc = which < 2, which % 2
            o_sb = s["qpT" if isq else "kpT"]
            nc.vector.tensor_add(
                out=o_sb[:, mc, :], in0=s.pop(f"pp{which}"), in1=pet[:, mc, :]
            )

        def unit_colsum(u):
            # column sums along k (partition axis) via ones matmul + recip
            if not (0 <= u < n_st):
                return
            s = st_state[u]
            sums = s["scps"][0:2, 4:8, :]
            for g in range(4):
                nc.tensor.matmul(
                    sums[:, g, :], ones_blk, s["attn"][:, g, :],
                    start=True, stop=True,
                )
            rec = sm.tile([2, 4, S], f32r, tag="rec", name="rec")
            with nc.allow_low_precision(reason="f32r output is full fp32 width"):
                nc.vector.reciprocal(rec, sums)
            s["rec"] = rec

        def unit_norm(u):
            # broadcast rec across partitions, masked per view parity:
            # recbcM[p, v, q] = rec[h(v), g(v), q] if p//64 == h(v) else 0,
            # then ab = attn * recbcM gives per-view attention weights that
            # are zero outside the view's partition half — so attn@v can
            # contract over the full 128 partitions with no tile_position.
            if not (0 <= u < n_st):
                return
            s = st_state[u]
            recbcM = psR.tile([128, 2, 4, S], fp32, tag="recbcM", name="recbcM")
            for e in range(2):
                nc.tensor.matmul(
                    recbcM[:, e, :, :].rearrange("p a b -> p (a b)"),
                    onesM[:, e, :],
                    s["rec"].rearrange("p a b -> p (a b)"),
                    start=True,
                    stop=True,
                )
            ab = sm.tile([128, 8, S], bf16, tag="ab", name="ab")
            nc.vector.tensor_tensor(
                ab.rearrange("p (a two) b -> p a two b", two=2),
                s["attn"][:, :, None, :].to_broadcast((128, 4, 2, S)),
                recbcM.rearrange("p two g q -> p g two q"),
                ALU.mult,
            )
            s["ab"] = ab

        def unit_fin_mm(w, mc):
            # final projection matmuls for stage w + allocate psf
            if not (0 <= w < n_st):
                return
            s = st_state[w]
            if mc == 0:
                s["pre"] = stg.tile([128, 2, R], fp32, tag="pre", name="pre")
            psf = psA.tile([128, R], fp32, tag="psA", name="ps_fin")
            for kc in range(2):
                nc.tensor.matmul(
                    psf,
                    wo[:, kc, mc * 128:(mc + 1) * 128],
                    s["outT"][:, kc, :],
                    start=(kc == 0),
                    stop=(kc == 1),
                )
            s[f"psf{mc}"] = psf

        def unit_fin_ev(w, mc):
            if not (0 <= w < n_st):
                return
            s = st_state[w]
            nc.scalar.copy(out=s["pre"][:, mc, :], in_=s.pop(f"psf{mc}"))

        def unit_av(u, c):
            # attn @ vp directly in transposed space outT[d, row] (PSUM only;
            # evacuation happens next iteration in unit_fc)
            if not (0 <= u < n_st):
                return
            s = st_state[u]
            pso = psB.tile([128, 8, S], fp32, tag="psB", name="ps_av")
            for v in range(NV):
                g = v // 2
                nc.tensor.matmul(
                    pso[:, v, :],
                    s["vp"][:, g, c * 128:(c + 1) * 128],
                    s["ab"][:, v, :],
                    start=True,
                    stop=True,
                )
            s[f"pso{c}"] = pso

        def unit_vp_mm(t, gg):
            # vp in natural [row, dout] layout (vt as stationary)
            if not (0 <= t < n_st):
                return
            s = st_state[t]
            if gg == 0:
                s["vp"] = proj.tile([128, 4, D], bf16, tag="vp", name="vp")
            psv = psB.tile([128, 2, D], fp32, tag="psB", name="ps_vp")
            for g2 in range(2):
                g = gg * 2 + g2
                for kc in range(2):
                    nc.tensor.matmul(
                        psv[:, g2, :],
                        s["vt"][:, kc, g * 128:(g + 1) * 128],
                        wv[:, kc, :],
                        start=(kc == 0),
                        stop=False,
                    )
                # pe add folded in as a matmul: E2.T @ pe tiles pe over rows
                nc.tensor.matmul(psv[:, g2, :], e2, pe_sb, start=False, stop=True)
            if gg == 0:
                # first chunk evacuated on DVE at end of this iteration
                s["psv0"] = psv
            else:
                # second chunk evacuated on ACT early next iteration
                s["psv1"] = psv

        def unit_vp0copy(t):
            if not (0 <= t < n_st):
                return
            s = st_state[t]
            nc.vector.tensor_copy(s["vp"][:, 0:2, :], s["psv0"])

        def unit_flush(sts):
            # gelu + residual + store for a completed group (Gelu-table period)
            for pst in sts:
                s = st_state[pst]
                go = proj.tile([128, 2, R], bf16, tag="go",
                               bufs=GELU_GROUP + 1, name="go")
                outsb = proj.tile([128, 2, R], bf16, tag="outsb",
                                  bufs=GELU_GROUP + 1, name="outsb")
                for mc in range(2):
                    _g = nc.scalar.activation(
                        out=go[:, mc, :], in_=s["pre"][:, mc, :],
                        func=AF.Gelu, bias=bo_sb[:, mc:mc + 1], scale=1.0,
                    )
                    if act_state["last_exp"] is not None:
                        add_dep_helper(_g.ins, act_state["last_exp"], sync=False,
                                       reason="act-table grouping: gelu after group exps")
                    act_state["last_gelu"] = _g.ins
                # residual add on GPSIMD (SBUF-only engine with slack);
                # tail supertiles split across Pool+DVE to shorten the drain
                if pst >= 12:
                    nc.gpsimd.tensor_add(
                        out=outsb[:, 0, :], in0=go[:, 0, :], in1=s["qt"][:, 0, :]
                    )
                    nc.vector.tensor_add(
                        out=outsb[:, 1, :], in0=go[:, 1, :], in1=s["qt"][:, 1, :]
                    )
                else:
                    nc.gpsimd.tensor_add(
                        out=outsb.rearrange("p a b -> p (a b)"),
                        in0=go.rearrange("p a b -> p (a b)"),
                        in1=s["qt"].rearrange("p a b -> p (a b)"),
                    )
                nc.sync.dma_start(
                    r3(out_d[:])[:, :, pst * R:(pst + 1) * R], outsb
                )
                del st_state[pst]

        # first input tiles right behind wq, then the remaining consts
        unit_load(0)
        pet = const.tile([128, 2, R], fp32)
        nc.sync.dma_start(pet, r3(pet_d[:]))
        nc.sync.dma_start(wk, r3(wk_d[:]))
        wv = const.tile([128, 2, D], bf16)
        wo = const.tile([128, 2, D], bf16)
        nc.sync.dma_start(wv, r3(wv_d[:]))
        nc.sync.dma_start(wo, r3(wo_d[:]))
        pe_sb = const.tile([S, D], bf16)
        nc.sync.dma_start(pe_sb, pe_d[:])
        e2 = const.tile([S, 128], bf16)
        nc.sync.dma_start(e2, e2_d[:])
        ones_blk = const.tile([128, 2], bf16)
        nc.sync.dma_start(ones_blk, ob_d[:])
        onesM = const.tile([2, 2, 128], f32r)
        nc.sync.dma_start(onesM, o2_d[:])
        bo_sb = const.tile([128, 2], fp32)
        nc.sync.dma_start(bo_sb, bo_d.rearrange("(kc p) -> p kc", p=128))

        if os.environ.get("KERNEL_NO_PIPELINE"):
            # debug mode: same units, strict per-supertile dataflow order;
            # KERNEL_STAGE=n truncates the per-supertile dataflow for bisection
            stage = int(os.environ.get("KERNEL_STAGE", "99"))
            for t in range(n_st):
                rs = slice(t * R, (t + 1) * R)
                unit_load(t + 1)
                for which in range(4):
                    unit_proj_mm(t, which)
                    unit_proj_ev(t, which)
                if stage <= 1:
                    nc.sync.dma_start(r3(out_d[:])[:, :, rs], st_state[t]["qpT"])
                    continue
                unit_vp_mm(t, 0)
                unit_vp0copy(t)
                unit_vp_mm(t, 1)
                unit_vp1copy(t)
                if stage <= 2:
                    nc.sync.dma_start(
                        r3(out_d[:])[:, 0, rs],
                        st_state[t]["vp"].rearrange("p a b -> p (a b)")[:, 0:512])
                    continue
                unit_scores(t)
                if stage <= 3:
                    nc.sync.dma_start(
                        r3(out_d[:])[:, 0, t * R: t * R + 256],
                        st_state[t]["attn"].rearrange("p a b -> p (a b)"))
                    continue
                unit_colsum(t)
                unit_norm(t)
                if stage <= 4:
                    nc.sync.dma_start(
                        r3(out_d[:])[:, 0, t * R: t * R + 512],
                        st_state[t]["ab"].rearrange("p a b -> p (a b)"))
                    continue
                unit_av(t, 0)
                unit_av(t, 1)
                unit_fc(t)
                if stage <= 5:
                    nc.sync.dma_start(r3(out_d[:])[:, :, rs], st_state[t]["outT"])
                    continue
                unit_fin_mm(t, 0)
                unit_fin_ev(t, 0)
                unit_fin_mm(t, 1)
                unit_fin_ev(t, 1)
                if (t + 1) % GELU_GROUP == 0:
                    g = (t + 1) // GELU_GROUP - 1
                    unit_flush(range(g * GELU_GROUP, (g + 1) * GELU_GROUP))
        # ---- software-pipelined main loop (two-stage skew) ----
        flush_at = {5: range(0, 4), 9: range(4, 8), 13: range(8, 12)}
        for t in ([] if os.environ.get("KERNEL_NO_PIPELINE") else range(n_st + 1)):
            u, w = t - 1, t - 2
            unit_load(t + 1)
            unit_scores(u)          # PE: scores | ACT: exp
            unit_fc(w)              # DVE: outT c0 | ACT: outT c1
            unit_vp1copy(t - 1)     # ACT: vp chunk 1 of t-1
            unit_proj_mm(t, 0)      # PE
            unit_proj_ev(t, 0)      # DVE
            unit_proj_mm(t, 1)
            unit_proj_ev(t, 1)      # DVE
            unit_proj_mm(t, 2)
            unit_proj_ev(t, 2)      # DVE
            unit_proj_mm(t, 3)
            unit_proj_ev(t, 3)      # DVE
            unit_fin_mm(w, 0)       # PE
            unit_fin_ev(w, 0)       # ACT
            unit_fin_mm(w, 1)
            unit_fin_ev(w, 1)       # ACT
            unit_colsum(u)          # PE colsum | DVE recip
            unit_vp_mm(t, 0)        # PE (fills recip latency)
            unit_vp0copy(t)         # DVE
            unit_norm(u)            # PE recbc | DVE mult
            unit_vp_mm(t, 1)        # PE (fills mult latency)
            unit_av(u, 0)           # PE
            unit_av(u, 1)           # PE
            if t == n_st:
                # drain: collapse the skew for the last supertile and run a
                # single contiguous gelu period for the remaining group
                unit_fc(u)
                unit_fin_mm(u, 0)
                unit_fin_ev(u, 0)
                unit_fin_mm(u, 1)
                unit_fin_ev(u, 1)
                unit_flush(range(12, 16))
            elif t in flush_at:
                unit_flush(flush_at[t])

    nc.finalize()
    return nc


def _get_nc():
    if "nc" not in _CACHE:
        _CACHE["nc"] = _build()
    return _CACHE["nc"]


def _host_inputs(q, k, v, Wq, Wk, Wv, Wo, bo):
    import ml_dtypes
    bf = ml_dtypes.bfloat16
    pe = _make_posenc(D, S)                      # [S, D] fp32
    peT_rep = np.ascontiguousarray(np.tile(pe.T, (1, NV)))   # [D, R] fp32
    e2 = np.ascontiguousarray(np.tile(np.eye(S, dtype=np.float32), (1, 2)))
    ones_blk = np.zeros((128, 2), np.float32)
    ones_blk[:64, 0] = 1.0
    ones_blk[64:, 1] = 1.0
    # onesM[h, e, m]: stationary for parity e selects rec row h==e into
    # output partitions m//64 == e (zero elsewhere)
    onesM = np.zeros((2, 2, 128), np.float32)
    onesM[0, 0, :64] = 1.0
    onesM[1, 1, 64:] = 1.0
    consts = {
        "WqT": np.ascontiguousarray(np.asarray(Wq, np.float32).T).astype(bf),
        "WkT": np.ascontiguousarray(np.asarray(Wk, np.float32).T).astype(bf),
        "WvT": np.ascontiguousarray(np.asarray(Wv, np.float32).T).astype(bf),
        "WoT": np.ascontiguousarray(np.asarray(Wo, np.float32).T).astype(bf),
        "bo": np.ascontiguousarray(np.asarray(bo, np.float32)),
        "peT_rep": peT_rep,
        "pe_nat": pe.astype(bf),
        "E2": e2.astype(bf),
        "ones_blk": ones_blk.astype(bf),
        "onesM": onesM,
    }
    in_maps = []
    for c in range(N_CORES):
        sl = slice(c * VC, (c + 1) * VC)
        m = dict(consts)
        m["qT"] = np.ascontiguousarray(
            np.asarray(q, np.float32)[sl].reshape(ROWS, D).T).astype(bf)
        m["kT"] = np.ascontiguousarray(
            np.asarray(k, np.float32)[sl].reshape(ROWS, D).T).astype(bf)
        m["vT"] = np.ascontiguousarray(
            np.asarray(v, np.float32)[sl].reshape(ROWS, D).T).astype(bf)
        in_maps.append(m)
    return in_maps


def kernel(q, k, v, Wq, Wk, Wv, Wo, bo, _trace=False):
    from concourse.bass_utils import run_bass_kernel_spmd

    nc = _get_nc()
    in_maps = _host_inputs(q, k, v, Wq, Wk, Wv, Wo, bo)
    res = run_bass_kernel_spmd(nc, in_maps, list(range(N_CORES)), trace=_trace)
    outs = [
        np.asarray(res.results[c]["outT"], dtype=np.float32)
        .reshape(D, VC, S).transpose(1, 2, 0)
        for c in range(N_CORES)
    ]
    full = np.concatenate(outs, axis=0).astype(np.float32)
    if _trace:
        _CACHE["last_results"] = res
    return full
